# revision 13
# baseline (speedup 1.0000x reference)
"""Block-circulant linear layer (y = x @ W^T + bias, W built from 64x64
circulant blocks) on 8 Trainium2 NeuronCores.

Math: per output block j, input block i: y[t,j] = sum_i circ(c[j,i]) @ x[t,i].
Via the convolution theorem this is, for each rfft bin k:
    Yhat[t,j,k] = sum_i Chat[j,i,k] * Xhat[t,i,k]   (complex)
i.e. 33 independent complex [64 x 64] matmuls over the block index, batched
over tokens. The host does the cheap O(T*F*logB) DFTs + layout packing; the
device does the dominant compute — the per-frequency complex matmuls.

Real/complex packing (per frequency k, contraction over rows r):
    rhs rows r:   [Xr_i (64) ; Xi_i (64)],  cols = tokens
    lhsT[i,    j] =  Cr[j,i]    lhsT[i,    64+j] = Ci[j,i]
    lhsT[64+i, j] = -Ci[j,i]    lhsT[64+i, 64+j] = Cr[j,i]
    out rows:     [Yr_j (64) ; Yi_j (64)]
Bins k=0 and k=32 are purely real (real input DFT), so they share one tile
(kt=0) with a block-diagonal lhsT; kt=1..31 carry bin k = kt.

Sharding: by frequency tile — core m owns k-tiles [4m, 4m+4) for ALL 4096
tokens. vs. token sharding this cuts the replicated c load 1MB -> 128KB per
core and turns the HBM streams into a handful of >=1MB DMAs (the kernel is
HBM-bound: ~8.1MB/core at ~360-430 GB/s; compute is ~7us of the ~25us floor).
"""

import numpy as np

_B = 64          # circulant block size
_NBLK = 64       # input/output blocks (4096/64)
_NK = 33         # rfft bins of a 64-point real signal
_NKT = 32        # packed frequency tiles (k0+k32 share tile 0)
_NCORES = 8
_KTC = _NKT // _NCORES   # k-tiles per core = 4
_T = 4096        # tokens = 2*2048
_F = 4096

_CACHE = {}

# matmul input precision: fp16 (vs fp32/fp32r) halves DMA bytes; accumulation
# is fp32 either way. Frequency-domain values are O(30) so fp16 rounding is
# ~3e-4 relative on the result — gate is 2e-2.
_IN_PREC = "fp16"
_OUT_PREC = "fp16"
_NP_IN = {"fp32r": np.float32, "fp16": np.float16}


def _build_cmat(c):
    """c: [J=64, I=64, B=64] float32 -> packed lhsT matrix [128, NKT*128]."""
    fc = np.fft.rfft(np.asarray(c, np.float32), axis=-1)  # [J, I, 33] complex64
    Cr, Ci = fc.real, fc.imag
    cm = np.zeros((_NKT, 128, 128), np.float32)  # [kt, row, col]
    cm[0, 0:64, 0:64] = Cr[:, :, 0].T
    cm[0, 64:128, 64:128] = Cr[:, :, 32].T
    for k in range(1, 32):
        cm[k, 0:64, 0:64] = Cr[:, :, k].T
        cm[k, 64:128, 0:64] = -Ci[:, :, k].T
        cm[k, 0:64, 64:128] = Ci[:, :, k].T
        cm[k, 64:128, 64:128] = Cr[:, :, k].T
    # device layout: [128 partitions, kt*128 + col]
    out = np.ascontiguousarray(cm.transpose(1, 0, 2)).reshape(128, _NKT * 128)
    return out.astype(_NP_IN[_IN_PREC])


def _build_xk(x):
    """x: [2, 2048, 4096] float32 -> packed rhs [NKT, 128, T]."""
    xb = np.asarray(x, np.float32).reshape(_T, _NBLK, _B)
    fx = np.fft.rfft(xb, axis=-1)            # [T, I, 33] complex64
    R = fx.real.transpose(2, 1, 0)           # [33, I, T]
    Im = fx.imag.transpose(2, 1, 0)
    XKf = np.empty((_NKT, 128, _T), np.float32)
    XKf[0, 0:64] = R[0]
    XKf[0, 64:128] = R[32]
    XKf[1:32, 0:64] = R[1:32]
    XKf[1:32, 64:128] = Im[1:32]
    return XKf


def _unpack_y(YKf, bias):
    """YKf: [NKT, 128, T] device output -> y [2, 2048, 4096] float32."""
    re = np.zeros((_NK, _NBLK, _T), np.float32)
    im = np.zeros((_NK, _NBLK, _T), np.float32)
    re[0] = YKf[0, 0:64]
    re[32] = YKf[0, 64:128]
    re[1:32] = YKf[1:32, 0:64]
    im[1:32] = YKf[1:32, 64:128]
    Yf = (re + 1j * im).transpose(2, 1, 0)   # [T, J, 33]
    yb = np.fft.irfft(Yf, n=_B, axis=-1).astype(np.float32)  # [T, J, B]
    y = yb.reshape(_T, _F) + np.asarray(bias, np.float32)
    return np.ascontiguousarray(y.reshape(2, _T // 2, _F))


def _build_device():
    import concourse.bacc as bacc
    import concourse.mybir as mybir
    import concourse.tile as tile

    f32 = mybir.dt.float32
    mmdt = {
        "fp32r": mybir.dt.float32r,
        "fp16": mybir.dt.float16,
    }[_IN_PREC]
    outdt = {"fp32": f32, "fp16": mybir.dt.float16}[_OUT_PREC]
    nc = bacc.Bacc("TRN2", target_bir_lowering=False, debug=False)
    # k-tile g of this core lives at cols [g*T, (g+1)*T)
    xk = nc.dram_tensor("xk", [128, _KTC * _T], mmdt, kind="ExternalInput")
    cm = nc.dram_tensor("cm", [128, _KTC * 128], mmdt, kind="ExternalInput")
    yk = nc.dram_tensor("yk", [128, _KTC * _T], outdt, kind="ExternalOutput")

    # Tile's epilogue clears every allocated semaphore (~254) serially on
    # GpSimd at ~28.5ns each = ~7.2us of teardown inside the measured exec
    # window. Split the clear across all five sequencers (it runs between
    # two all-engine barriers, so cross-engine clearing is race-free).
    from concourse.bass import compact_to_ranges

    orig_clear = nc.clear_and_free_semaphores

    def _split_clear(sems):
        nums = sorted(s.num if hasattr(s, "num") else s for s in sems)
        if len(nums) < 40:
            return orig_clear(nums)
        full = range(nums[0], nums[-1] + 1)
        nc.gpsimd.dma_reset(full)
        engines = [nc.gpsimd, nc.vector, nc.scalar, nc.tensor, nc.sync]
        per = (len(nums) + len(engines) - 1) // len(engines)
        for i, eng in enumerate(engines):
            chunk = nums[i * per:(i + 1) * per]
            for r in compact_to_ranges(chunk):
                eng.sem_clear(r)
        nc._state.prepend_free_semaphores(nums)
        for poison in nc._tile_sem_poison_stack:
            poison.update(nums)

    nc.clear_and_free_semaphores = _split_clear

    with tile.TileContext(nc) as tc:
        with (
            tc.tile_pool(name="cpool", bufs=1) as cpool,
            tc.tile_pool(name="xpool", bufs=1) as xpool,
            tc.tile_pool(name="ypool", bufs=1) as ypool,
            tc.tile_pool(name="pp", bufs=3, space="PSUM") as pp,
            tc.tile_pool(name="wpp", bufs=1, space="PSUM") as wpp,
        ):
            # all loads up front on the SP HWDGE ring: 128KB of weights, then
            # the x tiles. First and last tiles go as 512KB halves: a DMA's
            # completion sem fires ~2us after the data (write receipt), so a
            # small first chunk starts the matmul stream earlier and a small
            # last chunk shortens the end-of-pipeline latency.
            ct = cpool.tile([128, _KTC * 128], mmdt, tag="cw", name="cw")
            nc.sync.dma_start(out=ct[:], in_=cm[:, :])
            xts = []
            half = _T // 2
            for g in range(_KTC):
                xt = xpool.tile([128, _T], mmdt, tag=f"x{g}", name=f"x{g}")
                if g in (0, _KTC - 1):
                    for s in range(2):
                        nc.sync.dma_start(
                            out=xt[:, s * half:(s + 1) * half],
                            in_=xk[:, g * _T + s * half:g * _T + (s + 1) * half],
                        )
                else:
                    nc.sync.dma_start(out=xt[:], in_=xk[:, g * _T:(g + 1) * _T])
                xts.append(xt)
            # HAM warmup: dummy matmuls on memset scratch (NOT on ct — that
            # would chain the warmup behind the weight DMA's completion sem
            # and push the whole matmul stream ~3us later). These start as
            # soon as the memsets land, so the PE clock gate (1.2 -> 2.4 GHz
            # after ~3.4us of sustained activity) opens during the loads.
            wlhs = cpool.tile([128, 128], mmdt, tag="wlhs", name="wlhs")
            wrhs = cpool.tile([128, 512], mmdt, tag="wrhs", name="wrhs")
            nc.gpsimd.memset(wlhs[:], 0.0)
            nc.gpsimd.memset(wrhs[:], 0.0)
            wps = wpp.tile([128, 512], f32, name="wps")
            for _w in range(8):
                nc.tensor.matmul(
                    wps[:], lhsT=wlhs[:], rhs=wrhs[:], start=True, stop=True
                )
            # DVE copies are a touch slower per element and ACT also issues
            # the stores + its one-time table load: 10/6 split balances them
            act_copies = {1, 4, 6, 9, 12, 14}
            copy_idx = 0
            for g in range(_KTC):
                yt = ypool.tile([128, _T], outdt, tag=f"y{g}", name=f"y{g}")
                for h in range(4):
                    # 2-bank PSUM tile, two matmuls, one wide copy
                    ps = pp.tile([128, 1024], f32)
                    for jj in range(2):
                        c0 = h * 1024 + jj * 512
                        nc.tensor.matmul(
                            ps[:, jj * 512:(jj + 1) * 512],
                            lhsT=ct[:, g * 128:(g + 1) * 128],
                            rhs=xts[g][:, c0:c0 + 512],
                            start=True,
                            stop=True,
                        )
                    yslice = yt[:, h * 1024:(h + 1) * 1024]
                    if copy_idx in act_copies:
                        nc.scalar.copy(yslice, ps[:])
                    else:
                        nc.vector.tensor_copy(yslice, ps[:])
                    copy_idx += 1
                # stores on the ACT HWDGE ring; the last k-tile goes as two
                # 512KB halves so the final transfer on the critical tail is
                # short
                if g == _KTC - 1:
                    for s in range(2):
                        nc.scalar.dma_start(
                            out=yk[:, g * _T + s * half:g * _T + (s + 1) * half],
                            in_=yt[:, s * half:(s + 1) * half],
                        )
                else:
                    nc.scalar.dma_start(out=yk[:, g * _T:(g + 1) * _T], in_=yt[:])
    nc.compile()
    return nc


def _execute(in_maps, **kwargs):
    from concourse.bass_utils import run_bass_kernel_spmd

    if "nc" not in _CACHE:
        _CACHE["nc"] = _build_device()
    return run_bass_kernel_spmd(
        _CACHE["nc"], in_maps, core_ids=list(range(_NCORES)), **kwargs
    )


def _make_in_maps(x, c):
    XKf = _build_xk(x).astype(_NP_IN[_IN_PREC])  # [NKT, 128, T]
    cmd = _build_cmat(c)                         # [128, NKT*128]
    maps = []
    for m in range(_NCORES):
        xkm = (
            XKf[m * _KTC:(m + 1) * _KTC]         # [KTC, 128, T]
            .transpose(1, 0, 2)
            .reshape(128, _KTC * _T)
        )
        cmm = cmd[:, m * _KTC * 128:(m + 1) * _KTC * 128]
        maps.append(
            {
                "xk": np.ascontiguousarray(xkm),
                "cm": np.ascontiguousarray(cmm),
            }
        )
    return maps


def _gather_yk(results):
    """Per-core yk [128, KTC*T] -> full [NKT, 128, T]."""
    per_core = []
    for r in results:
        ykm = np.asarray(r["yk"]).reshape(128, _KTC, _T).transpose(1, 0, 2)
        per_core.append(ykm.astype(np.float32))
    return np.concatenate(per_core, axis=0)


def kernel(x, c, bias, **_kwargs):
    in_maps = _make_in_maps(x, c)
    bkr = _execute(in_maps)
    return _unpack_y(_gather_yk(bkr.results), bias)


# revision 14
# speedup vs baseline: 1.0153x; 1.0153x over previous
"""Block-circulant linear layer (y = x @ W^T + bias, W built from 64x64
circulant blocks) on 8 Trainium2 NeuronCores.

Math: per output block j, input block i: y[t,j] = sum_i circ(c[j,i]) @ x[t,i].
Via the convolution theorem this is, for each rfft bin k:
    Yhat[t,j,k] = sum_i Chat[j,i,k] * Xhat[t,i,k]   (complex)
i.e. 33 independent complex [64 x 64] matmuls over the block index, batched
over tokens. The host does the cheap O(T*F*logB) DFTs + layout packing; the
device does the dominant compute — the per-frequency complex matmuls.

Real/complex packing (per frequency k, contraction over rows r):
    rhs rows r:   [Xr_i (64) ; Xi_i (64)],  cols = tokens
    lhsT[i,    j] =  Cr[j,i]    lhsT[i,    64+j] = Ci[j,i]
    lhsT[64+i, j] = -Ci[j,i]    lhsT[64+i, 64+j] = Cr[j,i]
    out rows:     [Yr_j (64) ; Yi_j (64)]
Bins k=0 and k=32 are purely real (real input DFT), so they share one tile
(kt=0) with a block-diagonal lhsT; kt=1..31 carry bin k = kt.

Sharding: by frequency tile — core m owns k-tiles [4m, 4m+4) for ALL 4096
tokens. vs. token sharding this cuts the replicated c load 1MB -> 128KB per
core and turns the HBM streams into a handful of >=1MB DMAs (the kernel is
HBM-bound: ~8.1MB/core at ~360-430 GB/s; compute is ~7us of the ~25us floor).
"""

import numpy as np

_B = 64          # circulant block size
_NBLK = 64       # input/output blocks (4096/64)
_NK = 33         # rfft bins of a 64-point real signal
_NKT = 32        # packed frequency tiles (k0+k32 share tile 0)
_NCORES = 8
_KTC = _NKT // _NCORES   # k-tiles per core = 4
_T = 4096        # tokens = 2*2048
_F = 4096

_CACHE = {}

# matmul input precision: fp16 (vs fp32/fp32r) halves DMA bytes; accumulation
# is fp32 either way. Frequency-domain values are O(30) so fp16 rounding is
# ~3e-4 relative on the result — gate is 2e-2.
_IN_PREC = "fp16"
_OUT_PREC = "fp16"
_NP_IN = {"fp32r": np.float32, "fp16": np.float16}


def _build_cmat(c):
    """c: [J=64, I=64, B=64] float32 -> packed lhsT matrix [128, NKT*128]."""
    fc = np.fft.rfft(np.asarray(c, np.float32), axis=-1)  # [J, I, 33] complex64
    Cr, Ci = fc.real, fc.imag
    cm = np.zeros((_NKT, 128, 128), np.float32)  # [kt, row, col]
    cm[0, 0:64, 0:64] = Cr[:, :, 0].T
    cm[0, 64:128, 64:128] = Cr[:, :, 32].T
    for k in range(1, 32):
        cm[k, 0:64, 0:64] = Cr[:, :, k].T
        cm[k, 64:128, 0:64] = -Ci[:, :, k].T
        cm[k, 0:64, 64:128] = Ci[:, :, k].T
        cm[k, 64:128, 64:128] = Cr[:, :, k].T
    # device layout: [128 partitions, kt*128 + col]
    out = np.ascontiguousarray(cm.transpose(1, 0, 2)).reshape(128, _NKT * 128)
    return out.astype(_NP_IN[_IN_PREC])


def _build_xk(x):
    """x: [2, 2048, 4096] float32 -> packed rhs [NKT, 128, T]."""
    xb = np.asarray(x, np.float32).reshape(_T, _NBLK, _B)
    fx = np.fft.rfft(xb, axis=-1)            # [T, I, 33] complex64
    R = fx.real.transpose(2, 1, 0)           # [33, I, T]
    Im = fx.imag.transpose(2, 1, 0)
    XKf = np.empty((_NKT, 128, _T), np.float32)
    XKf[0, 0:64] = R[0]
    XKf[0, 64:128] = R[32]
    XKf[1:32, 0:64] = R[1:32]
    XKf[1:32, 64:128] = Im[1:32]
    return XKf


def _unpack_y(YKf, bias):
    """YKf: [NKT, 128, T] device output -> y [2, 2048, 4096] float32."""
    re = np.zeros((_NK, _NBLK, _T), np.float32)
    im = np.zeros((_NK, _NBLK, _T), np.float32)
    re[0] = YKf[0, 0:64]
    re[32] = YKf[0, 64:128]
    re[1:32] = YKf[1:32, 0:64]
    im[1:32] = YKf[1:32, 64:128]
    Yf = (re + 1j * im).transpose(2, 1, 0)   # [T, J, 33]
    yb = np.fft.irfft(Yf, n=_B, axis=-1).astype(np.float32)  # [T, J, B]
    y = yb.reshape(_T, _F) + np.asarray(bias, np.float32)
    return np.ascontiguousarray(y.reshape(2, _T // 2, _F))


def _build_device():
    import concourse.bacc as bacc
    import concourse.mybir as mybir
    import concourse.tile as tile

    f32 = mybir.dt.float32
    mmdt = {
        "fp32r": mybir.dt.float32r,
        "fp16": mybir.dt.float16,
    }[_IN_PREC]
    outdt = {"fp32": f32, "fp16": mybir.dt.float16}[_OUT_PREC]
    nc = bacc.Bacc("TRN2", target_bir_lowering=False, debug=False)
    # k-tile g of this core lives at cols [g*T, (g+1)*T)
    xk = nc.dram_tensor("xk", [128, _KTC * _T], mmdt, kind="ExternalInput")
    cm = nc.dram_tensor("cm", [128, _KTC * 128], mmdt, kind="ExternalInput")
    yk = nc.dram_tensor("yk", [128, _KTC * _T], outdt, kind="ExternalOutput")

    with tile.TileContext(nc) as tc:
        with (
            tc.tile_pool(name="cpool", bufs=1) as cpool,
            tc.tile_pool(name="xpool", bufs=1) as xpool,
            tc.tile_pool(name="ypool", bufs=1) as ypool,
            tc.tile_pool(name="pp", bufs=3, space="PSUM") as pp,
            tc.tile_pool(name="wpp", bufs=1, space="PSUM") as wpp,
        ):
            # all loads up front on the SP HWDGE ring: 128KB of weights, then
            # the 4 x 1MB rhs tiles (per-partition lines are 8KB contiguous)
            ct = cpool.tile([128, _KTC * 128], mmdt, tag="cw", name="cw")
            nc.sync.dma_start(out=ct[:], in_=cm[:, :])
            xts = []
            for g in range(_KTC):
                xt = xpool.tile([128, _T], mmdt, tag=f"x{g}", name=f"x{g}")
                nc.sync.dma_start(out=xt[:], in_=xk[:, g * _T:(g + 1) * _T])
                xts.append(xt)
            # HAM warmup: dummy matmuls on zeroed tiles while the first loads
            # are in flight, so the real matmul stream runs at full clock
            # instead of the cold 1.2 GHz gate.
            wlhs = cpool.tile([128, 128], mmdt, tag="wlhs", name="wlhs")
            wrhs = cpool.tile([128, 512], mmdt, tag="wrhs", name="wrhs")
            nc.gpsimd.memset(wlhs[:], 0.0)
            nc.gpsimd.memset(wrhs[:], 0.0)
            wps = wpp.tile([128, 512], f32, name="wps")
            for _w in range(10):
                nc.tensor.matmul(
                    wps[:], lhsT=wlhs[:], rhs=wrhs[:], start=True, stop=True
                )
            copy_idx = 0
            for g in range(_KTC):
                yt = ypool.tile([128, _T], outdt, tag=f"y{g}", name=f"y{g}")
                for h in range(4):
                    # 2-bank PSUM tile, two matmuls, one wide copy
                    ps = pp.tile([128, 1024], f32)
                    for jj in range(2):
                        c0 = h * 1024 + jj * 512
                        nc.tensor.matmul(
                            ps[:, jj * 512:(jj + 1) * 512],
                            lhsT=ct[:, g * 128:(g + 1) * 128],
                            rhs=xts[g][:, c0:c0 + 512],
                            start=True,
                            stop=True,
                        )
                    yslice = yt[:, h * 1024:(h + 1) * 1024]
                    # every 3rd wide copy goes to ACT, rest to DVE
                    if copy_idx % 3 == 2:
                        nc.scalar.copy(yslice, ps[:])
                    else:
                        nc.vector.tensor_copy(yslice, ps[:])
                    copy_idx += 1
                # 1MB store per k-tile on the ACT HWDGE ring
                nc.scalar.dma_start(out=yk[:, g * _T:(g + 1) * _T], in_=yt[:])
    nc.compile()
    return nc


def _execute(in_maps, **kwargs):
    from concourse.bass_utils import run_bass_kernel_spmd

    if "nc" not in _CACHE:
        _CACHE["nc"] = _build_device()
    return run_bass_kernel_spmd(
        _CACHE["nc"], in_maps, core_ids=list(range(_NCORES)), **kwargs
    )


def _make_in_maps(x, c):
    XKf = _build_xk(x).astype(_NP_IN[_IN_PREC])  # [NKT, 128, T]
    cmd = _build_cmat(c)                         # [128, NKT*128]
    maps = []
    for m in range(_NCORES):
        xkm = (
            XKf[m * _KTC:(m + 1) * _KTC]         # [KTC, 128, T]
            .transpose(1, 0, 2)
            .reshape(128, _KTC * _T)
        )
        cmm = cmd[:, m * _KTC * 128:(m + 1) * _KTC * 128]
        maps.append(
            {
                "xk": np.ascontiguousarray(xkm),
                "cm": np.ascontiguousarray(cmm),
            }
        )
    return maps


def _gather_yk(results):
    """Per-core yk [128, KTC*T] -> full [NKT, 128, T]."""
    per_core = []
    for r in results:
        ykm = np.asarray(r["yk"]).reshape(128, _KTC, _T).transpose(1, 0, 2)
        per_core.append(ykm.astype(np.float32))
    return np.concatenate(per_core, axis=0)


def kernel(x, c, bias, **_kwargs):
    in_maps = _make_in_maps(x, c)
    bkr = _execute(in_maps)
    return _unpack_y(_gather_yk(bkr.results), bias)


# revision 15
# speedup vs baseline: 1.0670x; 1.0508x over previous
"""Block-circulant linear layer (y = x @ W^T + bias, W built from 64x64
circulant blocks) on 8 Trainium2 NeuronCores.

Math: per output block j, input block i: y[t,j] = sum_i circ(c[j,i]) @ x[t,i].
Via the convolution theorem this is, for each rfft bin k:
    Yhat[t,j,k] = sum_i Chat[j,i,k] * Xhat[t,i,k]   (complex)
i.e. 33 independent complex [64 x 64] matmuls over the block index, batched
over tokens. The host does the cheap O(T*F*logB) DFTs + layout packing; the
device does the dominant compute — the per-frequency complex matmuls.

Real/complex packing (per frequency k, contraction over rows r):
    rhs rows r:   [Xr_i (64) ; Xi_i (64)],  cols = tokens
    lhsT[i,    j] =  Cr[j,i]    lhsT[i,    64+j] = Ci[j,i]
    lhsT[64+i, j] = -Ci[j,i]    lhsT[64+i, 64+j] = Cr[j,i]
    out rows:     [Yr_j (64) ; Yi_j (64)]
Bins k=0 and k=32 are purely real (real input DFT), so they share one tile
(kt=0) with a block-diagonal lhsT; kt=1..31 carry bin k = kt.

Sharding: by frequency tile — core m owns k-tiles [4m, 4m+4) for ALL 4096
tokens. vs. token sharding this cuts the replicated c load 1MB -> 128KB per
core and turns the HBM streams into a handful of >=1MB DMAs (the kernel is
HBM-bound: ~8.1MB/core at ~360-430 GB/s; compute is ~7us of the ~25us floor).
"""

import numpy as np

_B = 64          # circulant block size
_NBLK = 64       # input/output blocks (4096/64)
_NK = 33         # rfft bins of a 64-point real signal
_NKT = 32        # packed frequency tiles (k0+k32 share tile 0)
_NCORES = 8
_KTC = _NKT // _NCORES   # k-tiles per core = 4
_T = 4096        # tokens = 2*2048
_F = 4096

_CACHE = {}

# matmul input precision: fp16 (vs fp32/fp32r) halves DMA bytes; accumulation
# is fp32 either way. Frequency-domain values are O(30) so fp16 rounding is
# ~3e-4 relative on the result — gate is 2e-2.
_IN_PREC = "fp16"
_OUT_PREC = "fp16"
_NP_IN = {"fp32r": np.float32, "fp16": np.float16}


def _build_cmat(c):
    """c: [J=64, I=64, B=64] float32 -> packed lhsT matrix [128, NKT*128]."""
    fc = np.fft.rfft(np.asarray(c, np.float32), axis=-1)  # [J, I, 33] complex64
    Cr, Ci = fc.real, fc.imag
    cm = np.zeros((_NKT, 128, 128), np.float32)  # [kt, row, col]
    cm[0, 0:64, 0:64] = Cr[:, :, 0].T
    cm[0, 64:128, 64:128] = Cr[:, :, 32].T
    for k in range(1, 32):
        cm[k, 0:64, 0:64] = Cr[:, :, k].T
        cm[k, 64:128, 0:64] = -Ci[:, :, k].T
        cm[k, 0:64, 64:128] = Ci[:, :, k].T
        cm[k, 64:128, 64:128] = Cr[:, :, k].T
    # device layout: [128 partitions, kt*128 + col]
    out = np.ascontiguousarray(cm.transpose(1, 0, 2)).reshape(128, _NKT * 128)
    return out.astype(_NP_IN[_IN_PREC])


def _build_xk(x):
    """x: [2, 2048, 4096] float32 -> packed rhs [NKT, 128, T]."""
    xb = np.asarray(x, np.float32).reshape(_T, _NBLK, _B)
    fx = np.fft.rfft(xb, axis=-1)            # [T, I, 33] complex64
    R = fx.real.transpose(2, 1, 0)           # [33, I, T]
    Im = fx.imag.transpose(2, 1, 0)
    XKf = np.empty((_NKT, 128, _T), np.float32)
    XKf[0, 0:64] = R[0]
    XKf[0, 64:128] = R[32]
    XKf[1:32, 0:64] = R[1:32]
    XKf[1:32, 64:128] = Im[1:32]
    return XKf


def _unpack_y(YKf, bias):
    """YKf: [NKT, 128, T] device output -> y [2, 2048, 4096] float32."""
    re = np.zeros((_NK, _NBLK, _T), np.float32)
    im = np.zeros((_NK, _NBLK, _T), np.float32)
    re[0] = YKf[0, 0:64]
    re[32] = YKf[0, 64:128]
    re[1:32] = YKf[1:32, 0:64]
    im[1:32] = YKf[1:32, 64:128]
    Yf = (re + 1j * im).transpose(2, 1, 0)   # [T, J, 33]
    yb = np.fft.irfft(Yf, n=_B, axis=-1).astype(np.float32)  # [T, J, B]
    y = yb.reshape(_T, _F) + np.asarray(bias, np.float32)
    return np.ascontiguousarray(y.reshape(2, _T // 2, _F))


def _build_device():
    import concourse.bacc as bacc
    import concourse.mybir as mybir
    import concourse.tile as tile

    f32 = mybir.dt.float32
    mmdt = {
        "fp32r": mybir.dt.float32r,
        "fp16": mybir.dt.float16,
    }[_IN_PREC]
    outdt = {"fp32": f32, "fp16": mybir.dt.float16}[_OUT_PREC]
    nc = bacc.Bacc("TRN2", target_bir_lowering=False, debug=False)
    # k-tile g of this core lives at cols [g*T, (g+1)*T)
    xk = nc.dram_tensor("xk", [128, _KTC * _T], mmdt, kind="ExternalInput")
    cm = nc.dram_tensor("cm", [128, _KTC * 128], mmdt, kind="ExternalInput")
    yk = nc.dram_tensor("yk", [128, _KTC * _T], outdt, kind="ExternalOutput")

    with tile.TileContext(nc) as tc:
        with (
            tc.tile_pool(name="cpool", bufs=1) as cpool,
            tc.tile_pool(name="xpool", bufs=1) as xpool,
            tc.tile_pool(name="ypool", bufs=1) as ypool,
            tc.tile_pool(name="pp", bufs=3, space="PSUM") as pp,
            tc.tile_pool(name="wpp", bufs=1, space="PSUM") as wpp,
        ):
            # all loads up front on the SP HWDGE ring: 128KB of weights, then
            # the 4 x 1MB rhs tiles (per-partition lines are 8KB contiguous)
            ct = cpool.tile([128, _KTC * 128], mmdt, tag="cw", name="cw")
            nc.sync.dma_start(out=ct[:], in_=cm[:, :])
            xts = []
            for g in range(_KTC):
                xt = xpool.tile([128, _T], mmdt, tag=f"x{g}", name=f"x{g}")
                nc.sync.dma_start(out=xt[:], in_=xk[:, g * _T:(g + 1) * _T])
                xts.append(xt)
            # HAM warmup: dummy matmuls on zeroed tiles while the first loads
            # are in flight, so the real matmul stream runs at full clock
            # instead of the cold 1.2 GHz gate.
            wlhs = cpool.tile([128, 128], mmdt, tag="wlhs", name="wlhs")
            wrhs = cpool.tile([128, 512], mmdt, tag="wrhs", name="wrhs")
            nc.gpsimd.memset(wlhs[:], 0.0)
            nc.gpsimd.memset(wrhs[:], 0.0)
            wps = wpp.tile([128, 512], f32, name="wps")
            for _w in range(10):
                nc.tensor.matmul(
                    wps[:], lhsT=wlhs[:], rhs=wrhs[:], start=True, stop=True
                )
            copy_idx = 0
            for g in range(_KTC):
                yt = ypool.tile([128, _T], outdt, tag=f"y{g}", name=f"y{g}")
                last = g == _KTC - 1
                for h in range(4):
                    # 2-bank PSUM tile, two matmuls, one wide copy
                    ps = pp.tile([128, 1024], f32)
                    for jj in range(2):
                        c0 = h * 1024 + jj * 512
                        nc.tensor.matmul(
                            ps[:, jj * 512:(jj + 1) * 512],
                            lhsT=ct[:, g * 128:(g + 1) * 128],
                            rhs=xts[g][:, c0:c0 + 512],
                            start=True,
                            stop=True,
                        )
                    yslice = yt[:, h * 1024:(h + 1) * 1024]
                    # Every 3rd wide copy goes to ACT, rest to DVE — except
                    # the LAST k-tile, whose copy latency sits on the exec
                    # tail (its x arrives last): alternate engines there so
                    # its four copies drain pairwise in parallel.
                    if (h % 2 == 1) if last else (copy_idx % 3 == 2):
                        nc.scalar.copy(yslice, ps[:])
                    else:
                        nc.vector.tensor_copy(yslice, ps[:])
                    copy_idx += 1
                    if last and h == 1:
                        # first half of the last tile goes out as soon as its
                        # two copies land
                        nc.scalar.dma_start(
                            out=yk[:, g * _T:g * _T + _T // 2],
                            in_=yt[:, 0:_T // 2],
                        )
                # stores on the ACT HWDGE ring; the final transfer is a
                # 512KB half-tile so the end-of-kernel store+receipt is short
                if last:
                    nc.scalar.dma_start(
                        out=yk[:, g * _T + _T // 2:(g + 1) * _T],
                        in_=yt[:, _T // 2:_T],
                    )
                else:
                    nc.scalar.dma_start(
                        out=yk[:, g * _T:(g + 1) * _T], in_=yt[:]
                    )
    nc.compile()
    return nc


def _execute(in_maps, **kwargs):
    from concourse.bass_utils import run_bass_kernel_spmd

    if "nc" not in _CACHE:
        _CACHE["nc"] = _build_device()
    return run_bass_kernel_spmd(
        _CACHE["nc"], in_maps, core_ids=list(range(_NCORES)), **kwargs
    )


def _make_in_maps(x, c):
    XKf = _build_xk(x).astype(_NP_IN[_IN_PREC])  # [NKT, 128, T]
    cmd = _build_cmat(c)                         # [128, NKT*128]
    maps = []
    for m in range(_NCORES):
        xkm = (
            XKf[m * _KTC:(m + 1) * _KTC]         # [KTC, 128, T]
            .transpose(1, 0, 2)
            .reshape(128, _KTC * _T)
        )
        cmm = cmd[:, m * _KTC * 128:(m + 1) * _KTC * 128]
        maps.append(
            {
                "xk": np.ascontiguousarray(xkm),
                "cm": np.ascontiguousarray(cmm),
            }
        )
    return maps


def _gather_yk(results):
    """Per-core yk [128, KTC*T] -> full [NKT, 128, T]."""
    per_core = []
    for r in results:
        ykm = np.asarray(r["yk"]).reshape(128, _KTC, _T).transpose(1, 0, 2)
        per_core.append(ykm.astype(np.float32))
    return np.concatenate(per_core, axis=0)


def kernel(x, c, bias, **_kwargs):
    in_maps = _make_in_maps(x, c)
    bkr = _execute(in_maps)
    return _unpack_y(_gather_yk(bkr.results), bias)
